# revision 13
# baseline (speedup 1.0000x reference)
"""Hough-transform voting kernel for Trainium2 (8 NeuronCores), v5.

out[m, b] = (1/128) * sum_i w_i * x[m, p_i] * [bin_i == b],  m in 0..31 maps,
b in 0..33119 bins, 4M votes, votes sharded 8 ways across NeuronCores.

Structure (per core, votes sharded round-robin):
  - cell grid: cell = (brange of 256 bins, pixel block of 128 pixels),
    cap 32 votes; 4 cells per 128-slot chunk; 32 cell chunks + 3 SWDGE
    spill chunks per brange (NCH=35). Cell overflow votes (~5%) are
    gathered via one batched SWDGE dma_gather per 8 branges.
  - gather: one matmul per cell: psum[32 slots, 32 maps] = OH^T @ xblock,
    OH built on DVE as is_equal(pix_replicated, iotapF) — all operands
    innermost-packed so the DVE 2x mode applies.
  - scatter uses a factored one-hot: bin_local = 2*hi + lo (hi in 0..127,
    lo in 0..1). Mhi[s, hi] one-hot on DVE (2x); W2[s, lo, m] =
    v[s, m] * wc[s] * [bllo[s] == lo] built on GPSIMD; per chunk one
    matmul psum[128 hi, (lo, m)] += Mhi^T @ W2 — 35 matmuls of N=64
    per brange, 4x less PE scatter time than a 256-wide mask and 25%
    less one-hot build work.
  - psum->SBUF copies run on the otherwise-idle ACT engine.
  - output: [128 hi, NBR * 2 * 32] f32 partials; host sums 8 cores and
    reindexes bin = r*256 + hi*2 + lo.
"""

import numpy as np

IM_H, IM_W = 128, 128
HT_H, HT_W = 184, 180
NB = HT_H * HT_W          # 33120 bins
NPIX = IM_H * IM_W        # 16384 pixels
NMAPS = 32
NCORES = 8
NORM = 128.0

BR_BINS = 256             # bins per brange
NBR = (NB + BR_BINS - 1) // BR_BINS       # 130
NBLK = NPIX // 128        # 128 pixel blocks
CELL_CAP = 32
CELLS_PER_CHUNK = 4       # 128 slots per cell chunk, psum offsets 0/32/64/96
NCELL_CHUNKS = NBLK // CELLS_PER_CHUNK    # 32 (128 cells, no dummy)
SPILL_CHUNKS = 3          # 384 spill slots per brange (128 each)
NCH = NCELL_CHUNKS + SPILL_CHUNKS         # 35 chunks per brange
CSLOTS = NCELL_CHUNKS * 128               # 4096 cell slots per brange
SSLOTS = SPILL_CHUNKS * 128               # 384 spill slots per brange
DUMMY_PIX = 200.0
DUMMY_HI = 300.0
DUMMY_LO = 3.0


def _build_shard(vp, vb, vw):
    """Lay one shard's votes into the slot grid.

    Returns (pixloc [NBR*CSLOTS] f32, wc/blhi/bllo [128, NBR*NCH] f32,
             spill_idx [16, NBR*SSLOTS/16] i16)."""
    r = vb >> 8
    blk = vp >> 7
    cell = r * NBLK + blk
    order = np.argsort(cell, kind="stable")
    cell_s = cell[order]
    vp_s = vp[order]
    vb_s = vb[order]
    w_s = vw[order] / NORM
    counts = np.bincount(cell_s, minlength=NBR * NBLK)
    starts = np.concatenate([[0], np.cumsum(counts)])
    within = np.arange(len(cell_s)) - starts[cell_s]

    nchunk = NBR * NCH
    wc = np.zeros((128, nchunk), np.float32)
    blhi = np.full((128, nchunk), DUMMY_HI, np.float32)
    bllo = np.full((128, nchunk), DUMMY_LO, np.float32)
    pixloc = np.full(NBR * CSLOTS, DUMMY_PIX, np.float32)
    spill_idx = np.zeros((16, NBR * SSLOTS // 16), np.int16)

    # primary (cell) votes
    prim = within < CELL_CAP
    rp = cell_s[prim] // NBLK
    bp = cell_s[prim] % NBLK
    wi = within[prim]
    binrel = vb_s[prim] - rp * BR_BINS
    ch = rp * NCH + bp // CELLS_PER_CHUNK          # global chunk
    srow = (bp % CELLS_PER_CHUNK) * CELL_CAP + wi  # slot row in chunk (0..127)
    wc[srow, ch] = w_s[prim]
    blhi[srow, ch] = (binrel >> 1).astype(np.float32)
    bllo[srow, ch] = (binrel & 1).astype(np.float32)
    pixloc[rp * CSLOTS + (bp // CELLS_PER_CHUNK) * 128 + srow] = (
        vp_s[prim] & 127
    ).astype(np.float32)

    # spill votes: rank within brange
    sp = ~prim
    rs = cell_s[sp] // NBLK
    sp_counts = np.bincount(rs, minlength=NBR)
    if sp_counts.max() > SSLOTS:
        raise RuntimeError(f"spill overflow: {sp_counts.max()} > {SSLOTS}")
    sp_starts = np.concatenate([[0], np.cumsum(sp_counts)])
    j = np.arange(len(rs)) - sp_starts[rs]         # spill rank in brange
    binrel = vb_s[sp] - rs * BR_BINS
    ch = rs * NCH + NCELL_CHUNKS + j // 128
    srow = j % 128
    wc[srow, ch] = w_s[sp]
    blhi[srow, ch] = (binrel >> 1).astype(np.float32)
    bllo[srow, ch] = (binrel & 1).astype(np.float32)
    spill_idx[j % 16, rs * (SSLOTS // 16) + j // 16] = vp_s[sp].astype(np.int16)
    return pixloc, wc, blhi, bllo, spill_idx


def _prep_inputs(**inputs):
    import concourse.mybir as mybir

    bf16 = mybir.dt.np(mybir.dt.bfloat16)

    x = np.asarray(inputs["x"]).astype(np.float32)
    vp = np.asarray(inputs["vote_pixel"]).astype(np.int64)
    vb = np.asarray(inputs["vote_bin"]).astype(np.int64)
    vw = np.asarray(inputs["vote_weight"]).astype(np.float32)
    b, c = x.shape[0], x.shape[1]
    xf = x.reshape(b * c, NPIX)  # [32, 16384]

    # xb[p_local, block, map] for the PE gather; xt rows for the spill gather
    xb = np.ascontiguousarray(
        xf.T.reshape(NBLK, 128, NMAPS).transpose(1, 0, 2)
    ).astype(bf16)
    xt = np.zeros((NPIX, 128), bf16)
    xt[:, :NMAPS] = xf.T.astype(bf16)

    iotapF = np.broadcast_to(
        np.arange(128, dtype=np.float32).reshape(128, 1), (128, CSLOTS)
    ).astype(bf16)
    iotaA = np.broadcast_to(
        np.arange(128, dtype=np.float32).reshape(1, 128, 1), (128, 128, NCH)
    ).astype(bf16)
    iota2 = np.broadcast_to(
        np.arange(2, dtype=np.float32).reshape(1, 2, 1), (128, 2, NCH)
    ).astype(bf16)

    in_maps = []
    for s in range(NCORES):
        pixloc, wc, blhi, bllo, spill_idx = _build_shard(
            vp[s::NCORES], vb[s::NCORES], vw[s::NCORES]
        )
        idx_tile = np.concatenate([spill_idx, spill_idx], axis=0)  # HW reads 16..31
        in_maps.append({
            "xb": xb, "xt": xt,
            "pix": pixloc.reshape(1, -1).astype(bf16),
            "wc": wc.astype(bf16),
            "blhi": blhi.astype(bf16),
            "bllo": bllo.astype(bf16),
            "idx": idx_tile.astype(np.int16),
            "iotapF": np.ascontiguousarray(iotapF),
            "iotaA": np.ascontiguousarray(iotaA),
            "iota2": np.ascontiguousarray(iota2),
        })
    return in_maps, b, c


def _build_program():
    import concourse.bacc as bacc
    import concourse.mybir as mybir
    import concourse.tile as tile

    eq = mybir.AluOpType.is_equal
    mu = mybir.AluOpType.mult

    nc = bacc.Bacc("TRN2", target_bir_lowering=False, debug=False)
    nchunk = NBR * NCH
    xb_d = nc.dram_tensor("xb", [128, NBLK, NMAPS], mybir.dt.bfloat16,
                          kind="ExternalInput")
    xt_d = nc.dram_tensor("xt", [NPIX, 128], mybir.dt.bfloat16, kind="ExternalInput")
    pix_d = nc.dram_tensor("pix", [1, NBR * CSLOTS], mybir.dt.bfloat16,
                           kind="ExternalInput")
    wc_d = nc.dram_tensor("wc", [128, nchunk], mybir.dt.bfloat16,
                          kind="ExternalInput")
    blhi_d = nc.dram_tensor("blhi", [128, nchunk], mybir.dt.bfloat16,
                            kind="ExternalInput")
    bllo_d = nc.dram_tensor("bllo", [128, nchunk], mybir.dt.bfloat16,
                            kind="ExternalInput")
    idx_d = nc.dram_tensor("idx", [32, NBR * SSLOTS // 16], mybir.dt.int16,
                           kind="ExternalInput")
    iotapF_d = nc.dram_tensor("iotapF", [128, CSLOTS], mybir.dt.bfloat16,
                              kind="ExternalInput")
    iotaA_d = nc.dram_tensor("iotaA", [128, 128, NCH], mybir.dt.bfloat16,
                             kind="ExternalInput")
    iota2_d = nc.dram_tensor("iota2", [128, 2, NCH], mybir.dt.bfloat16,
                             kind="ExternalInput")
    ht_d = nc.dram_tensor("ht", [128, NBR * 2 * NMAPS], mybir.dt.float32,
                          kind="ExternalOutput")

    with tile.TileContext(nc) as tc:
        with (
            tc.tile_pool(name="st", bufs=1) as stp,
            tc.tile_pool(name="px", bufs=3) as pxp,
            tc.tile_pool(name="oh", bufs=3) as ohp,
            tc.tile_pool(name="mb", bufs=3) as mbp,
            tc.tile_pool(name="mk", bufs=3) as mkp,
            tc.tile_pool(name="ml", bufs=4) as mlp,
            tc.tile_pool(name="vs", bufs=2) as vsp,
            tc.tile_pool(name="vv", bufs=4) as vvp,
            tc.tile_pool(name="vw", bufs=4) as vwp,
            tc.tile_pool(name="ht", bufs=2) as htp,
            tc.tile_pool(name="pg", bufs=4, space="PSUM") as psg,
            tc.tile_pool(name="po", bufs=2, space="PSUM") as pso,
        ):
            xb_sb = stp.tile([128, NBLK, NMAPS], mybir.dt.bfloat16)
            nc.sync.dma_start(xb_sb[:], xb_d[:])
            iotapF_sb = stp.tile([128, CSLOTS], mybir.dt.bfloat16)
            nc.sync.dma_start(iotapF_sb[:], iotapF_d[:])
            iotaA_sb = stp.tile([128, 128, NCH], mybir.dt.bfloat16)
            nc.sync.dma_start(iotaA_sb[:], iotaA_d[:])
            iota2_sb = stp.tile([128, 2, NCH], mybir.dt.bfloat16)
            nc.sync.dma_start(iota2_sb[:], iota2_d[:])

            vsp8 = None
            wc8 = hi8 = lo8 = None
            strip = None
            opsum = None
            for r in range(NBR):
                # pixel row replicated across partitions; one-hot build on DVE
                # (packed operands -> 2x mode)
                pix_sb = pxp.tile([128, CSLOTS], mybir.dt.bfloat16, tag="px")
                nc.sync.dma_start(
                    pix_sb[:],
                    pix_d[:, r * CSLOTS:(r + 1) * CSLOTS].to_broadcast(
                        [128, CSLOTS]
                    ),
                )
                oh_sb = ohp.tile([128, CSLOTS], mybir.dt.bfloat16, tag="oh")
                nc.vector.tensor_tensor(
                    out=oh_sb[:], in0=pix_sb[:], in1=iotapF_sb[:], op=eq,
                )
                if r % 8 == 0:
                    # spill gather: one batched SWDGE gather per 8-brange
                    # octave (per-call fixed cost dominates small gathers)
                    n_oct = min(8, NBR - r)
                    osl = n_oct * SSLOTS
                    vsp8 = vsp.tile([128, 8 * SPILL_CHUNKS, 128],
                                    mybir.dt.bfloat16, tag="sp")
                    idx_sb = vsp.tile([32, 8 * SSLOTS // 16], mybir.dt.int16,
                                      tag="ix")
                    nc.sync.dma_start(
                        idx_sb[:, 0:osl // 16],
                        idx_d[:, r * (SSLOTS // 16):
                              r * (SSLOTS // 16) + osl // 16],
                    )
                    nc.gpsimd.dma_gather(
                        vsp8[:, 0:osl // 128, :], xt_d[:],
                        idx_sb[:, 0:osl // 16], osl, osl, 128,
                        single_packet=False,
                    )
                    # weights / bin-locals batched per octave
                    wc8 = mbp.tile([128, 8 * NCH], mybir.dt.bfloat16, tag="wc")
                    nc.sync.dma_start(
                        wc8[:, 0:n_oct * NCH],
                        wc_d[:, r * NCH:(r + n_oct) * NCH],
                    )
                    hi8 = mbp.tile([128, 8 * NCH], mybir.dt.bfloat16, tag="hi")
                    nc.sync.dma_start(
                        hi8[:, 0:n_oct * NCH],
                        blhi_d[:, r * NCH:(r + n_oct) * NCH],
                    )
                    lo8 = mbp.tile([128, 8 * NCH], mybir.dt.bfloat16, tag="lo")
                    nc.sync.dma_start(
                        lo8[:, 0:n_oct * NCH],
                        bllo_d[:, r * NCH:(r + n_oct) * NCH],
                    )
                ro = r % 8
                # scatter one-hots: Mhi [slots, 128, ch] on DVE (2x);
                # wcMlo [slots, 2, ch] = wc * [bllo == lo]
                mhi_sb = mkp.tile([128, 128, NCH], mybir.dt.bfloat16, tag="mh")
                nc.vector.tensor_tensor(
                    out=mhi_sb[:],
                    in0=hi8[:, ro * NCH:(ro + 1) * NCH].unsqueeze(1)
                        .to_broadcast([128, 128, NCH]),
                    in1=iotaA_sb[:],
                    op=eq,
                )
                mlo_sb = mlp.tile([128, 2, NCH], mybir.dt.bfloat16, tag="ml")
                nc.vector.tensor_tensor(
                    out=mlo_sb[:],
                    in0=lo8[:, ro * NCH:(ro + 1) * NCH].unsqueeze(1)
                        .to_broadcast([128, 2, NCH]),
                    in1=iota2_sb[:],
                    op=eq,
                )
                wm_sb = mlp.tile([128, 2, NCH], mybir.dt.bfloat16, tag="wm")
                nc.vector.tensor_tensor(
                    out=wm_sb[:],
                    in0=mlo_sb[:],
                    in1=wc8[:, ro * NCH:(ro + 1) * NCH].unsqueeze(1)
                        .to_broadcast([128, 2, NCH]),
                    op=mu,
                )
                # PE gather: one matmul per cell; psum -> v_sb on ACT with a
                # transposed view so v_sb is [slots, map, chunk] (packed chunk)
                v_sb = vvp.tile([128, NMAPS, NCH], mybir.dt.bfloat16, tag="v")
                for qb in range((NCELL_CHUNKS + 15) // 16):
                    nch_b = min(16, NCELL_CHUNKS - qb * 16)
                    psum = psg.tile([128, 16, NMAPS], mybir.dt.float32,
                                    space="PSUM")
                    for ci in range(nch_b):
                        ch = qb * 16 + ci
                        for seg in range(CELLS_PER_CHUNK):
                            blk = ch * CELLS_PER_CHUNK + seg
                            off = ch * 128 + seg * CELL_CAP
                            nc.tensor.matmul(
                                psum[seg * CELL_CAP:(seg + 1) * CELL_CAP,
                                     ci, :],
                                lhsT=oh_sb[:, off:off + CELL_CAP],
                                rhs=xb_sb[:, blk, :],
                                start=True, stop=True,
                                tile_position=(0, 96) if seg == 3 else None,
                            )
                    nc.scalar.copy(
                        out=v_sb[:, :, qb * 16:qb * 16 + nch_b],
                        in_=psum[:, 0:nch_b, :].transpose([0, 2, 1]),
                    )
                # spill values into v_sb tail chunks (ACT, transposed view)
                nc.scalar.copy(
                    out=v_sb[:, :, NCELL_CHUNKS:NCH],
                    in_=vsp8[:, ro * SPILL_CHUNKS:(ro + 1) * SPILL_CHUNKS,
                             0:NMAPS].transpose([0, 2, 1]),
                )
                # W2[s, lo, m, ch] = v * wcMlo on GPSIMD
                w2_sb = vwp.tile([128, 2, NMAPS, NCH], mybir.dt.bfloat16,
                                 tag="w2")
                nc.gpsimd.tensor_tensor(
                    out=w2_sb[:],
                    in0=v_sb[:].unsqueeze(1).to_broadcast(
                        [128, 2, NMAPS, NCH]),
                    in1=wm_sb[:].unsqueeze(2).to_broadcast(
                        [128, 2, NMAPS, NCH]),
                    op=mu,
                )
                # factored scatter: psum[128 hi, (lo, m)] += Mhi^T @ W2,
                # accumulated over the 35 chunks; 8 branges per psum bank
                if ro == 0:
                    opsum = pso.tile([128, 512], mybir.dt.float32,
                                     space="PSUM")
                base = ro * 2 * NMAPS
                for ch in range(NCH):
                    nc.tensor.matmul(
                        opsum[:, base:base + 2 * NMAPS],
                        lhsT=mhi_sb[:, :, ch].squeeze(),
                        rhs=w2_sb[:, :, :, ch].squeeze(),
                        start=(ch == 0), stop=(ch == NCH - 1),
                    )
                if ro == 7 or r == NBR - 1:
                    n_r = ro + 1
                    strip = htp.tile([128, 512], mybir.dt.float32, tag="ht")
                    nc.scalar.copy(
                        out=strip[:, 0:n_r * 64], in_=opsum[:, 0:n_r * 64],
                    )
                    o0 = (r - n_r + 1) * 64
                    nc.sync.dma_start(
                        ht_d[:, o0:o0 + n_r * 64], strip[:, 0:n_r * 64],
                    )
    nc.compile()
    return nc


def kernel(**inputs):
    from concourse import bass_utils

    in_maps, b, c = _prep_inputs(**inputs)

    global _PROG_CACHE
    try:
        cached = _PROG_CACHE
    except NameError:
        cached = _PROG_CACHE = {}
    globals()["_LAST_IN_MAPS"] = in_maps
    key = (NBR, NCH)
    if key not in cached:
        cached[key] = _build_program()
    nc = cached[key]
    res = bass_utils.run_bass_kernel_spmd(nc, in_maps, core_ids=list(range(NCORES)))
    return _combine(res, b, c)


def _combine(res, b, c):
    acc = np.zeros((128, NBR * 2 * NMAPS), np.float64)
    for s in range(NCORES):
        acc += res.results[s]["ht"].astype(np.float64)
    # [hi(128), brange, lo(2), map] -> bin = r*256 + hi*2 + lo
    part = acc.reshape(128, NBR, 2, NMAPS).transpose(1, 0, 2, 3)
    out = part.reshape(NBR * BR_BINS, NMAPS)[:NB].astype(np.float32)
    return np.ascontiguousarray(out.T).reshape(b, c, HT_H, HT_W)


# revision 16
# speedup vs baseline: 1.2886x; 1.2886x over previous
"""Hough-transform voting kernel for Trainium2 (8 NeuronCores), v5.

out[m, b] = (1/128) * sum_i w_i * x[m, p_i] * [bin_i == b],  m in 0..31 maps,
b in 0..33119 bins, 4M votes, votes sharded 8 ways across NeuronCores.

Structure (per core, votes sharded round-robin):
  - cell grid: cell = (brange of 256 bins, pixel block of 128 pixels),
    cap 32 votes; 4 cells per 128-slot chunk; 32 cell chunks + 3 SWDGE
    spill chunks per brange (NCH=35). Cell overflow votes (~5%) are
    gathered via one batched SWDGE dma_gather per 8 branges.
  - gather: one matmul per cell: psum[32 slots, 32 maps] = OH^T @ xblock,
    OH built on DVE as is_equal(pix_replicated, iotapF) — all operands
    innermost-packed so the DVE 2x mode applies.
  - scatter uses a factored one-hot: bin_local = 2*hi + lo (hi in 0..127,
    lo in 0..1). Mhi[s, hi] one-hot on DVE (2x); W2[s, lo, m] =
    v[s, m] * wc[s] * [bllo[s] == lo] built on GPSIMD; per chunk one
    matmul psum[128 hi, (lo, m)] += Mhi^T @ W2 — 35 matmuls of N=64
    per brange, 4x less PE scatter time than a 256-wide mask and 25%
    less one-hot build work.
  - psum->SBUF copies run on the otherwise-idle ACT engine.
  - output: [128 hi, NBR * 2 * 32] f32 partials; host sums 8 cores and
    reindexes bin = r*256 + hi*2 + lo.
"""

import numpy as np

IM_H, IM_W = 128, 128
HT_H, HT_W = 184, 180
NB = HT_H * HT_W          # 33120 bins
NPIX = IM_H * IM_W        # 16384 pixels
NMAPS = 32
NCORES = 8
NORM = 128.0

BR_BINS = 256             # bins per brange
NBR = (NB + BR_BINS - 1) // BR_BINS       # 130
NBLK = NPIX // 128        # 128 pixel blocks
CELL_CAP = 32
CELLS_PER_CHUNK = 4       # 128 slots per cell chunk, psum offsets 0/32/64/96
NCELL_CHUNKS = NBLK // CELLS_PER_CHUNK    # 32 (128 cells, no dummy)
SPILL_CHUNKS = 3          # 384 spill slots per brange (128 each)
NCH = NCELL_CHUNKS + SPILL_CHUNKS         # 35 chunks per brange
CSLOTS = NCELL_CHUNKS * 128               # 4096 cell slots per brange
SSLOTS = SPILL_CHUNKS * 128               # 384 spill slots per brange
DUMMY_PIX = 200.0
DUMMY_HI = 300.0
DUMMY_LO = 3.0


def _build_shard(vp, vb, vw):
    """Lay one shard's votes into the slot grid.

    Returns (pixloc [NBR*CSLOTS] f32, wc/blhi/bllo [128, NBR*NCH] f32,
             spill_idx [16, NBR*SSLOTS/16] i16)."""
    r = vb >> 8
    blk = vp >> 7
    cell = r * NBLK + blk
    order = np.argsort(cell, kind="stable")
    cell_s = cell[order]
    vp_s = vp[order]
    vb_s = vb[order]
    w_s = vw[order] / NORM
    counts = np.bincount(cell_s, minlength=NBR * NBLK)
    starts = np.concatenate([[0], np.cumsum(counts)])
    within = np.arange(len(cell_s)) - starts[cell_s]

    nchunk = NBR * NCH
    wc = np.zeros((128, nchunk), np.float32)
    blhi = np.full((128, nchunk), DUMMY_HI, np.float32)
    bllo = np.full((128, nchunk), DUMMY_LO, np.float32)
    pixloc = np.full(NBR * CSLOTS, DUMMY_PIX, np.float32)
    spill_idx = np.zeros((16, NBR * SSLOTS // 16), np.int16)

    # primary (cell) votes
    prim = within < CELL_CAP
    rp = cell_s[prim] // NBLK
    bp = cell_s[prim] % NBLK
    wi = within[prim]
    binrel = vb_s[prim] - rp * BR_BINS
    ch = rp * NCH + bp // CELLS_PER_CHUNK          # global chunk
    srow = (bp % CELLS_PER_CHUNK) * CELL_CAP + wi  # slot row in chunk (0..127)
    wc[srow, ch] = w_s[prim]
    blhi[srow, ch] = (binrel >> 1).astype(np.float32)
    bllo[srow, ch] = (binrel & 1).astype(np.float32)
    pixloc[rp * CSLOTS + (bp // CELLS_PER_CHUNK) * 128 + srow] = (
        vp_s[prim] & 127
    ).astype(np.float32)

    # spill votes: rank within brange
    sp = ~prim
    rs = cell_s[sp] // NBLK
    sp_counts = np.bincount(rs, minlength=NBR)
    if sp_counts.max() > SSLOTS:
        raise RuntimeError(f"spill overflow: {sp_counts.max()} > {SSLOTS}")
    sp_starts = np.concatenate([[0], np.cumsum(sp_counts)])
    j = np.arange(len(rs)) - sp_starts[rs]         # spill rank in brange
    binrel = vb_s[sp] - rs * BR_BINS
    ch = rs * NCH + NCELL_CHUNKS + j // 128
    srow = j % 128
    wc[srow, ch] = w_s[sp]
    blhi[srow, ch] = (binrel >> 1).astype(np.float32)
    bllo[srow, ch] = (binrel & 1).astype(np.float32)
    spill_idx[j % 16, rs * (SSLOTS // 16) + j // 16] = vp_s[sp].astype(np.int16)
    return pixloc, wc, blhi, bllo, spill_idx


def _prep_inputs(**inputs):
    import concourse.mybir as mybir

    bf16 = mybir.dt.np(mybir.dt.bfloat16)

    x = np.asarray(inputs["x"]).astype(np.float32)
    vp = np.asarray(inputs["vote_pixel"]).astype(np.int64)
    vb = np.asarray(inputs["vote_bin"]).astype(np.int64)
    vw = np.asarray(inputs["vote_weight"]).astype(np.float32)
    b, c = x.shape[0], x.shape[1]
    xf = x.reshape(b * c, NPIX)  # [32, 16384]

    # xb[p_local, block, map] for the PE gather; xt rows for the spill gather
    xb = np.ascontiguousarray(
        xf.T.reshape(NBLK, 128, NMAPS).transpose(1, 0, 2)
    ).astype(bf16)
    xt = np.zeros((NPIX, 128), bf16)
    xt[:, :NMAPS] = xf.T.astype(bf16)

    iotapF = np.broadcast_to(
        np.arange(128, dtype=np.float32).reshape(128, 1), (128, CSLOTS)
    ).astype(bf16)
    iotaA = np.broadcast_to(
        np.arange(128, dtype=np.float32).reshape(1, 128, 1), (128, 128, NCH)
    ).astype(bf16)
    iota2 = np.broadcast_to(
        np.arange(2, dtype=np.float32).reshape(1, 2, 1), (128, 2, NCH)
    ).astype(bf16)

    in_maps = []
    for s in range(NCORES):
        pixloc, wc, blhi, bllo, spill_idx = _build_shard(
            vp[s::NCORES], vb[s::NCORES], vw[s::NCORES]
        )
        idx_tile = np.concatenate([spill_idx, spill_idx], axis=0)  # HW reads 16..31
        in_maps.append({
            "xb": xb, "xt": xt,
            "pix": pixloc.reshape(1, -1).astype(bf16),
            "wc": wc.astype(bf16),
            "blhi": blhi.astype(bf16),
            "bllo": bllo.astype(bf16),
            "idx": idx_tile.astype(np.int16),
            "iotapF": np.ascontiguousarray(iotapF),
            "iotaA": np.ascontiguousarray(iotaA),
            "iota2": np.ascontiguousarray(iota2),
        })
    return in_maps, b, c


def _build_program():
    import concourse.bacc as bacc
    import concourse.mybir as mybir
    import concourse.tile as tile

    eq = mybir.AluOpType.is_equal
    mu = mybir.AluOpType.mult

    nc = bacc.Bacc("TRN2", target_bir_lowering=False, debug=False)
    nchunk = NBR * NCH
    xb_d = nc.dram_tensor("xb", [128, NBLK, NMAPS], mybir.dt.bfloat16,
                          kind="ExternalInput")
    xt_d = nc.dram_tensor("xt", [NPIX, 128], mybir.dt.bfloat16, kind="ExternalInput")
    pix_d = nc.dram_tensor("pix", [1, NBR * CSLOTS], mybir.dt.bfloat16,
                           kind="ExternalInput")
    wc_d = nc.dram_tensor("wc", [128, nchunk], mybir.dt.bfloat16,
                          kind="ExternalInput")
    blhi_d = nc.dram_tensor("blhi", [128, nchunk], mybir.dt.bfloat16,
                            kind="ExternalInput")
    bllo_d = nc.dram_tensor("bllo", [128, nchunk], mybir.dt.bfloat16,
                            kind="ExternalInput")
    idx_d = nc.dram_tensor("idx", [32, NBR * SSLOTS // 16], mybir.dt.int16,
                           kind="ExternalInput")
    iotapF_d = nc.dram_tensor("iotapF", [128, CSLOTS], mybir.dt.bfloat16,
                              kind="ExternalInput")
    iotaA_d = nc.dram_tensor("iotaA", [128, 128, NCH], mybir.dt.bfloat16,
                             kind="ExternalInput")
    iota2_d = nc.dram_tensor("iota2", [128, 2, NCH], mybir.dt.bfloat16,
                             kind="ExternalInput")
    ht_d = nc.dram_tensor("ht", [128, NBR * 2 * NMAPS], mybir.dt.float32,
                          kind="ExternalOutput")

    with tile.TileContext(nc) as tc:
        with (
            tc.tile_pool(name="st", bufs=1) as stp,
            tc.tile_pool(name="px", bufs=3) as pxp,
            tc.tile_pool(name="oh", bufs=3) as ohp,
            tc.tile_pool(name="mb", bufs=3) as mbp,
            tc.tile_pool(name="mk", bufs=3) as mkp,
            tc.tile_pool(name="ml", bufs=4) as mlp,
            tc.tile_pool(name="vs", bufs=2) as vsp,
            tc.tile_pool(name="vv", bufs=4) as vvp,
            tc.tile_pool(name="vw", bufs=4) as vwp,
            tc.tile_pool(name="ht", bufs=2) as htp,
            tc.tile_pool(name="pg", bufs=4, space="PSUM") as psg,
            tc.tile_pool(name="po", bufs=2, space="PSUM") as pso,
        ):
            xb_sb = stp.tile([128, NBLK, NMAPS], mybir.dt.bfloat16)
            nc.sync.dma_start(xb_sb[:], xb_d[:])
            iotapF_sb = stp.tile([128, CSLOTS], mybir.dt.bfloat16)
            nc.sync.dma_start(iotapF_sb[:], iotapF_d[:])
            iotaA_sb = stp.tile([128, 128, NCH], mybir.dt.bfloat16)
            nc.sync.dma_start(iotaA_sb[:], iotaA_d[:])
            iota2_sb = stp.tile([128, 2, NCH], mybir.dt.bfloat16)
            nc.sync.dma_start(iota2_sb[:], iota2_d[:])

            vsp8 = None
            wc8 = hi8 = lo8 = None
            strip = None
            opsum = None
            for r in range(NBR):
                # pixel row replicated across partitions; one-hot build on DVE
                # (packed operands -> 2x mode)
                pix_sb = pxp.tile([128, CSLOTS], mybir.dt.bfloat16, tag="px")
                nc.sync.dma_start(
                    pix_sb[:],
                    pix_d[:, r * CSLOTS:(r + 1) * CSLOTS].to_broadcast(
                        [128, CSLOTS]
                    ),
                )
                oh_sb = ohp.tile([128, CSLOTS], mybir.dt.bfloat16, tag="oh")
                nc.vector.tensor_tensor(
                    out=oh_sb[:], in0=pix_sb[:], in1=iotapF_sb[:], op=eq,
                )
                if r % 8 == 0:
                    # spill gather: one batched SWDGE gather per 8-brange
                    # octave (per-call fixed cost dominates small gathers)
                    n_oct = min(8, NBR - r)
                    osl = n_oct * SSLOTS
                    vsp8 = vsp.tile([128, 8 * SPILL_CHUNKS, 128],
                                    mybir.dt.bfloat16, tag="sp")
                    idx_sb = vsp.tile([32, 8 * SSLOTS // 16], mybir.dt.int16,
                                      tag="ix")
                    nc.sync.dma_start(
                        idx_sb[:, 0:osl // 16],
                        idx_d[:, r * (SSLOTS // 16):
                              r * (SSLOTS // 16) + osl // 16],
                    )
                    nc.gpsimd.dma_gather(
                        vsp8[:, 0:osl // 128, :], xt_d[:],
                        idx_sb[:, 0:osl // 16], osl, osl, 128,
                        single_packet=False,
                    )
                    # weights / bin-locals batched per octave
                    wc8 = mbp.tile([128, 8 * NCH], mybir.dt.bfloat16, tag="wc")
                    nc.sync.dma_start(
                        wc8[:, 0:n_oct * NCH],
                        wc_d[:, r * NCH:(r + n_oct) * NCH],
                    )
                    hi8 = mbp.tile([128, 8 * NCH], mybir.dt.bfloat16, tag="hi")
                    nc.sync.dma_start(
                        hi8[:, 0:n_oct * NCH],
                        blhi_d[:, r * NCH:(r + n_oct) * NCH],
                    )
                    lo8 = mbp.tile([128, 8 * NCH], mybir.dt.bfloat16, tag="lo")
                    nc.sync.dma_start(
                        lo8[:, 0:n_oct * NCH],
                        bllo_d[:, r * NCH:(r + n_oct) * NCH],
                    )
                ro = r % 8
                # scatter one-hots: Mhi [slots, 128, ch] on DVE (2x);
                # wcMlo [slots, 2, ch] = wc * [bllo == lo]
                mhi_sb = mkp.tile([128, 128, NCH], mybir.dt.bfloat16, tag="mh")
                nc.vector.tensor_tensor(
                    out=mhi_sb[:],
                    in0=hi8[:, ro * NCH:(ro + 1) * NCH].unsqueeze(1)
                        .to_broadcast([128, 128, NCH]),
                    in1=iotaA_sb[:],
                    op=eq,
                )
                mlo_sb = mlp.tile([128, 2, NCH], mybir.dt.bfloat16, tag="ml")
                nc.vector.tensor_tensor(
                    out=mlo_sb[:],
                    in0=lo8[:, ro * NCH:(ro + 1) * NCH].unsqueeze(1)
                        .to_broadcast([128, 2, NCH]),
                    in1=iota2_sb[:],
                    op=eq,
                )
                wm_sb = mlp.tile([128, 2, NCH], mybir.dt.bfloat16, tag="wm")
                nc.vector.tensor_tensor(
                    out=wm_sb[:],
                    in0=mlo_sb[:],
                    in1=wc8[:, ro * NCH:(ro + 1) * NCH].unsqueeze(1)
                        .to_broadcast([128, 2, NCH]),
                    op=mu,
                )
                # PE gather: one matmul per cell; psum -> v_sb on ACT with a
                # transposed view so v_sb is [slots, map, chunk] (packed chunk)
                v_sb = vvp.tile([128, NMAPS, NCH], mybir.dt.bfloat16, tag="v")
                for qb in range((NCELL_CHUNKS + 15) // 16):
                    nch_b = min(16, NCELL_CHUNKS - qb * 16)
                    psum = psg.tile([128, 16, NMAPS], mybir.dt.float32,
                                    space="PSUM")
                    for ci in range(nch_b):
                        ch = qb * 16 + ci
                        for seg in range(CELLS_PER_CHUNK):
                            blk = ch * CELLS_PER_CHUNK + seg
                            off = ch * 128 + seg * CELL_CAP
                            nc.tensor.matmul(
                                psum[seg * CELL_CAP:(seg + 1) * CELL_CAP,
                                     ci, :],
                                lhsT=oh_sb[:, off:off + CELL_CAP],
                                rhs=xb_sb[:, blk, :],
                                start=True, stop=True,
                                tile_position=(0, 96) if seg == 3 else None,
                            )
                    nc.scalar.copy(
                        out=v_sb[:, :, qb * 16:qb * 16 + nch_b],
                        in_=psum[:, 0:nch_b, :].transpose([0, 2, 1]),
                    )
                # spill values into v_sb tail chunks (ACT, transposed view)
                nc.scalar.copy(
                    out=v_sb[:, :, NCELL_CHUNKS:NCH],
                    in_=vsp8[:, ro * SPILL_CHUNKS:(ro + 1) * SPILL_CHUNKS,
                             0:NMAPS].transpose([0, 2, 1]),
                )
                # W2[s, lo, m, ch] = v * wcMlo on GPSIMD
                w2_sb = vwp.tile([128, 2, NMAPS, NCH], mybir.dt.bfloat16,
                                 tag="w2")
                nc.gpsimd.tensor_tensor(
                    out=w2_sb[:],
                    in0=v_sb[:].unsqueeze(1).to_broadcast(
                        [128, 2, NMAPS, NCH]),
                    in1=wm_sb[:].unsqueeze(2).to_broadcast(
                        [128, 2, NMAPS, NCH]),
                    op=mu,
                )
                # factored scatter: psum[128 hi, (lo, m)] += Mhi^T @ W2,
                # accumulated over the 35 chunks; 8 branges per psum bank
                if ro == 0:
                    opsum = pso.tile([128, 512], mybir.dt.float32,
                                     space="PSUM")
                base = ro * 2 * NMAPS
                for ch in range(NCH):
                    nc.tensor.matmul(
                        opsum[:, base:base + 2 * NMAPS],
                        lhsT=mhi_sb[:, :, ch].squeeze(),
                        rhs=w2_sb[:, :, :, ch].squeeze(),
                        start=(ch == 0), stop=(ch == NCH - 1),
                    )
                if ro == 7 or r == NBR - 1:
                    n_r = ro + 1
                    strip = htp.tile([128, 512], mybir.dt.float32, tag="ht")
                    nc.scalar.copy(
                        out=strip[:, 0:n_r * 64], in_=opsum[:, 0:n_r * 64],
                    )
                    o0 = (r - n_r + 1) * 64
                    nc.sync.dma_start(
                        ht_d[:, o0:o0 + n_r * 64], strip[:, 0:n_r * 64],
                    )
    nc.compile()
    return nc


def kernel(**inputs):
    from concourse import bass_utils

    in_maps, b, c = _prep_inputs(**inputs)

    global _PROG_CACHE
    try:
        cached = _PROG_CACHE
    except NameError:
        cached = _PROG_CACHE = {}
    globals()["_LAST_IN_MAPS"] = in_maps
    key = (NBR, NCH)
    if key not in cached:
        cached[key] = _build_program()
    nc = cached[key]
    res = bass_utils.run_bass_kernel_spmd(nc, in_maps, core_ids=list(range(NCORES)))
    return _combine(res, b, c)


def _combine(res, b, c):
    acc = np.zeros((128, NBR * 2 * NMAPS), np.float64)
    for s in range(NCORES):
        acc += res.results[s]["ht"].astype(np.float64)
    # [hi(128), brange, lo(2), map] -> bin = r*256 + hi*2 + lo
    part = acc.reshape(128, NBR, 2, NMAPS).transpose(1, 0, 2, 3)
    out = part.reshape(NBR * BR_BINS, NMAPS)[:NB].astype(np.float32)
    return np.ascontiguousarray(out.T).reshape(b, c, HT_H, HT_W)
